# revision 1
# baseline (speedup 1.0000x reference)
"""Trainium2 Bass kernel for the Backflow module (nn_Backflow_79809082294809).

Contract: kernel(**inputs) takes FULL unsharded inputs (numpy), returns the
FULL output [512, 32, 3] float32. Internally shards the batch dim across 8
NeuronCores (pure data parallel), runs one SPMD Bass/Tile kernel, gathers.

Math (per batch b, electron i):
  out = rs + 1e-4 * cutoff * (bf_elec + bf_nuc)
  bf_elec_i = sum_j w(i,j) * (r_i - r_j)   [full NxN pairs: diagonal cancels]
  bf_nuc_i  = sum_k wn(k)  * (r_i - c_k)
Both reduce to:  rs_i * T3 - T_c  with
  T = S + Tn + badd,  S[c',i] = sum_j W[j,i] * G[b,j,c'],  G=[rs|1]
  Tn = (nw3 @ [coords|1])^T h2n + [coords|1]^T nb3,  badd = eb3 * sum_j G
The shifted softplus ssp(x) = log(0.5 e^x + 0.5) is computed exactly as
Ln(0.5 * Exp(x) + 0.5) on the ACT engine (2 passes, one table set).
"""

import numpy as np
import ml_dtypes

import concourse.bacc as bacc
import concourse.mybir as mybir
import concourse.tile as tile
from concourse.bass_utils import run_bass_kernel_spmd

# Both Exp and Ln live in natural_log_exp_and_others; putting it first stops
# the act-table-load pass from alternating between the exp-only and ln-only
# sets (one table load instead of one per activation).
_orig_get_act_tables = bacc.get_activation_tables


def _patched_get_act_tables(arch):
    # Keep dict order/length (act_func_set_id is positional) but hide Exp/Ln
    # from the single-function sets so the first set matching either is
    # natural_log_exp_and_others, which holds both -> one table load total.
    t = dict(_orig_get_act_tables(arch))
    key = "natural_log_exp_and_others"
    if key in t:
        full = t[key]
        t = {k: (v if k == key else (set(v) - full)) for k, v in t.items()}
    return t


bacc.get_activation_tables = _patched_get_act_tables

F32 = mybir.dt.float32
BF16 = mybir.dt.bfloat16
F32R = mybir.dt.float32r
EXP = mybir.ActivationFunctionType.Exp
LN = mybir.ActivationFunctionType.Ln
IDENT = mybir.ActivationFunctionType.Identity

N_CORES = 8
B, N, D, K = 512, 32, 256, 8
CUTOFF_L = 0.5


# ---------------------------------------------------------------- host prep

def _host_prep(rs, xs, coords, ew1, eb1, ew2, eb2, ew3, eb3,
               nw1, nb1, nw2, nb2, nw3, nb3):
    """Build per-core input maps (all float32 numpy)."""
    rs = np.asarray(rs, np.float32)
    xs = np.asarray(xs, np.float32)
    coords = np.asarray(coords, np.float32)
    ew1 = np.asarray(ew1, np.float32)
    eb1 = np.asarray(eb1, np.float32)
    ew2 = np.asarray(ew2, np.float32)
    eb2 = np.asarray(eb2, np.float32)
    ew3 = np.asarray(ew3, np.float32)
    eb3 = np.asarray(eb3, np.float32)
    nw1 = np.asarray(nw1, np.float32)
    nb1 = np.asarray(nb1, np.float32)
    nw2 = np.asarray(nw2, np.float32)
    nb2 = np.asarray(nb2, np.float32)
    nw3 = np.asarray(nw3, np.float32)
    nb3 = np.asarray(nb3, np.float32)

    bc = B // N_CORES

    # xsT: [D, B*N] feature-major
    xsT = np.ascontiguousarray(xs.reshape(B * N, D).T)

    # G = [rs | 1] -> G2 per core [N, bc*4] (j-partition, (b,c') free)
    G = np.concatenate([rs, np.ones((B, N, 1), np.float32)], axis=2)  # [B,N,4]

    # cutoff (host): r = dist/L; f = where(r < L, r^2(6-8r+3r^2), 1); prod_k
    diffs = rs[:, :, None, :] - coords[None, None, :, :]          # [B,N,K,3]
    dist = np.sqrt((diffs * diffs).sum(-1).astype(np.float32))    # [B,N,K]
    r = (dist / np.float32(CUTOFF_L)).astype(np.float32)
    f = np.where(r < np.float32(CUTOFF_L),
                 r * r * (6.0 - 8.0 * r + 3.0 * r * r), np.float32(1.0))
    cutoff = f.astype(np.float32).prod(axis=-1)                   # [B,N]
    sc = (1e-4 * cutoff).astype(np.float32)                       # [B,N]

    # badd[b, i*4+c'] = eb3 * sum_j G[b,j,c']  (replicated over i)
    gsum = G.sum(axis=1) * np.float32(eb3[0])                     # [B,4]
    badd = np.tile(gsum, (1, N)).astype(np.float32)               # [B,4*N] (i,c')

    # --- packed / padded weights ---
    # mm1: lhsT [128, 64] per chunk; packed side by side -> [128, 128]
    ew1p = np.zeros((128, 128), np.float32)
    ew1p[:, 0:40] = ew1[0:128]
    ew1p[:, 64:104] = ew1[128:256]
    # mm2: K=64 strips at base {0,64}; M=32 padded
    ew2p = np.zeros((128, 32), np.float32)
    ew2p[0:40, 0:6] = ew2
    ew2p[64:104, 0:6] = ew2
    # mm3: K=32 strips at base {0,32,64,96}; M=32 padded (col 0 only)
    ew3p = np.zeros((128, 32), np.float32)
    for r4 in range(4):
        ew3p[32 * r4:32 * r4 + 6, 0] = ew3[:, 0]
    # nuc mm1: [128, 162] two chunks side by side
    nw1p = np.zeros((128, 176), np.float32)
    nw1p[:, 0:81] = nw1[0:128]
    nw1p[:, 88:169] = nw1[128:256]
    # nuc mm2: K=81 base 0; M=32 padded
    nw2p = np.zeros((81, 32), np.float32)
    nw2p[:, 0:25] = nw2
    # nuc mm3 folded with coords: nw3C [25,4]; K=32 strips; M=32 padded
    C = np.concatenate([coords, np.ones((K, 1), np.float32)], axis=1)  # [8,4]
    nw3C = (nw3 @ C).astype(np.float32)                                # [25,4]
    nw3Cp = np.zeros((128, 32), np.float32)
    for r4 in range(4):
        nw3Cp[32 * r4:32 * r4 + 25, 0:4] = nw3C
    CbT = (nb3 @ C).astype(np.float32)                                 # [4]

    # biases [128, 6]: col0 b1-packed, col1 b2-packed, col2 nb1,
    #                  col3 nb2-packed, col4 CbT-packed
    bia = np.zeros((128, 6), np.float32)
    bia[:, 5] = 0.5
    bia[0:40, 0] = eb1
    bia[64:104, 0] = eb1
    for r4 in range(4):
        bia[32 * r4:32 * r4 + 6, 1] = eb2
    bia[0:25, 3] = nb2
    bia[0:4, 4] = CbT
    bia[0:81, 2] = nb1

    in_maps = []
    for c in range(N_CORES):
        b0, b1_ = c * bc, (c + 1) * bc
        G2 = np.ascontiguousarray(
            G[b0:b1_].transpose(1, 0, 2).reshape(N, bc * 4))
        in_maps.append({
            "xsT": np.ascontiguousarray(xsT[:, b0 * N:b1_ * N]),
            "G2": G2,
            "rsf": np.ascontiguousarray(rs[b0:b1_].reshape(bc, N * 3)),
            "sc": np.ascontiguousarray(sc[b0:b1_]),
            "badd": np.ascontiguousarray(badd[b0:b1_]),
            "ew1p": ew1p.astype(ml_dtypes.bfloat16),
            "ew2p": ew2p.astype(ml_dtypes.bfloat16),
            "ew3p": ew3p.astype(ml_dtypes.bfloat16),
            "nw1p": nw1p,
            "nw2p": nw2p.astype(ml_dtypes.bfloat16),
            "nw3Cp": nw3Cp.astype(ml_dtypes.bfloat16),
            "bia": bia,
            "eye4": np.eye(4, dtype=np.float32),
        })
    return in_maps


# ---------------------------------------------------------------- bass build

def build_kernel(bc):
    """Build the per-core Bass module; bc = batches per core."""
    nc = bacc.Bacc("TRN2", target_bir_lowering=False, debug=False)

    cols = bc * N                     # (b,i) columns on this core
    gn = min(512, cols)               # nuc col-group size
    ngr = cols // gn                  # nuc groups (4 at bc=64)

    xsT = nc.dram_tensor("xsT", [D, cols], F32R, kind="ExternalInput")
    G2d = nc.dram_tensor("G2", [N, bc * 4], F32, kind="ExternalInput")
    rsfd = nc.dram_tensor("rsf", [bc, N * 3], F32, kind="ExternalInput")
    scd = nc.dram_tensor("sc", [bc, N], F32, kind="ExternalInput")
    baddd = nc.dram_tensor("badd", [bc, 4 * N], F32, kind="ExternalInput")
    ew1d = nc.dram_tensor("ew1p", [128, 128], BF16, kind="ExternalInput")
    ew2d = nc.dram_tensor("ew2p", [128, 32], BF16, kind="ExternalInput")
    ew3d = nc.dram_tensor("ew3p", [128, 32], BF16, kind="ExternalInput")
    nw1d = nc.dram_tensor("nw1p", [128, 176], F32R, kind="ExternalInput")
    nw2d = nc.dram_tensor("nw2p", [81, 32], BF16, kind="ExternalInput")
    nw3d = nc.dram_tensor("nw3Cp", [128, 32], BF16, kind="ExternalInput")
    biad = nc.dram_tensor("bia", [128, 6], F32, kind="ExternalInput")
    eyed = nc.dram_tensor("eye4", [4, 4], F32, kind="ExternalInput")
    outd = nc.dram_tensor("out", [bc, N * 3], F32, kind="ExternalOutput")
    tsd = nc.dram_tensor("tsd", [4, bc * N], F32)

    with tile.TileContext(nc) as tc:
        with tc.tile_pool(name="consts", bufs=1) as cp:
            ew1t = cp.tile([128, 128], BF16, name="ew1t")
            nc.sync.dma_start(ew1t[:], ew1d[:])
            ew2t = cp.tile([128, 32], BF16, name="ew2t")
            nc.sync.dma_start(ew2t[:], ew2d[:])
            ew3t = cp.tile([128, 32], BF16, name="ew3t")
            nc.sync.dma_start(ew3t[:], ew3d[:])
            nw1t = cp.tile([128, 176], F32R, name="nw1t")
            nc.sync.dma_start(nw1t[:], nw1d[:])
            nw2t = cp.tile([81, 32], BF16, name="nw2t")
            nc.sync.dma_start(nw2t[:], nw2d[:])
            nw3t = cp.tile([128, 32], BF16, name="nw3t")
            nc.sync.dma_start(nw3t[:], nw3d[:])
            biat = cp.tile([128, 6], F32, name="biat")
            nc.sync.dma_start(biat[:], biad[:])
            eyet = cp.tile([4, 4], F32, name="eyet")
            nc.sync.dma_start(eyet[:], eyed[:])
            G2t = cp.tile([N, bc * 4], F32, name="G2t")
            nc.sync.dma_start(G2t[:], G2d[:])
            xt0 = cp.tile([128, cols], F32R, name="xt0")
            xt1 = cp.tile([128, cols], F32R, name="xt1")
            UBL = min(8, bc) * N
            for q in range(cols // UBL):
                qs = slice(q * UBL, (q + 1) * UBL)
                nc.sync.dma_start(xt0[:, qs], xsT[0:128, qs])
                nc.sync.dma_start(xt1[:, qs], xsT[128:256, qs])
            Wt = cp.tile([N, cols], F32, name="Wt")
            TS = cp.tile([4, cols], F32, name="TS")

            Tn4 = cp.tile([4, cols], F32, name="Tn4")
            h1n = cp.tile([81, cols], BF16, name="h1n")

            # ---------------- shared pools: nuc MLP + e-e pipeline ------
            UB = min(8, bc)
            with tc.tile_pool(name="eps", bufs=3, space="PSUM") as eps, \
                 tc.tile_pool(name="ewk", bufs=4) as ewk, \
                 tc.tile_pool(name="sps", bufs=1, space="PSUM") as sps:
                # nucleus MLP, one col-group at a time; emitted interleaved
                # into the unit loop so its serial chain fills bubbles
                nps = eps
                nwk = ewk

                def nuc_group(g):
                    gs = slice(g * gn, (g + 1) * gn)
                    psn1 = nps.tile([128, gn], F32, name="psn1",
                                    tag="ps1")[0:81, :]
                    nc.tensor.matmul(psn1[:], nw1t[:, 0:81], xt0[:, gs],
                                     start=True, stop=False)
                    nc.tensor.matmul(psn1[:], nw1t[:, 88:169], xt1[:, gs],
                                     start=False, stop=True)
                    nc.scalar.activation(psn1[:], psn1[:], EXP,
                                         bias=biat[0:81, 2:3])
                    nc.scalar.activation(h1n[:, gs], psn1[:], LN,
                                         bias=biat[0:81, 5:6], scale=0.5)
                    psn2 = nps.tile([128, gn], F32, name="psn2",
                                    tag="ps2", bufs=2)[0:32, :]
                    nc.tensor.matmul(psn2[:], nw2t[:], h1n[:, gs],
                                     start=True, stop=True)
                    nc.scalar.activation(psn2[:], psn2[:], EXP,
                                         bias=biat[0:32, 3:4])
                    h2g = nwk.tile([128, gn], BF16, name="h2g",
                                   tag="h1")[0:32, :]
                    nc.scalar.activation(h2g[:], psn2[:], LN,
                                         bias=biat[0:32, 5:6], scale=0.5)
                    psn3 = nps.tile([128, gn], F32, name="psn3",
                                    tag="ps3", bufs=2)[0:32, :]
                    nc.tensor.matmul(psn3[:], nw3t[0:32, :], h2g[:],
                                     start=True, stop=True)
                    nc.vector.tensor_scalar_add(Tn4[:, gs], psn3[0:4, :],
                                                biat[0:4, 4:5])

                # ---------------- electron-electron pipeline -------------
                # pair-column order per 8-batch unit: (j, b, i) so the W
                # repack scatters in 1KB runs instead of 128B
                for g in range(ngr):
                    nuc_group(g)
                for u in range(bc // UB):
                    c0 = u * UB * N
                    uc = UB * N * N
                    pt0 = ewk.tile([128, uc], BF16, name="pt0", tag="pt0", bufs=2)
                    pt1 = ewk.tile([128, uc], BF16, name="pt1", tag="pt1", bufs=2)
                    # split pair products DVE:GPSIMD ~ 5:3 (GPSIMD runs
                    # 2-input elementwise at about half the DVE rate)
                    for ci, (xt, pt) in enumerate(((xt0, pt0), (xt1, pt1))):
                        xj = xt[:, c0:c0 + UB * N].rearrange(
                            "p (b j) -> p j b", b=UB)[:, :, :, None]
                        xi = xt[:, c0:c0 + UB * N].rearrange(
                            "p (b i) -> p b i", b=UB)[:, None, :, :]
                        xjb = xj.broadcast_to([128, N, UB, N])
                        xib = xi.broadcast_to([128, N, UB, N])
                        ptv = pt.rearrange("p (j b i) -> p j b i", j=N, b=UB)
                        if ci == 0:
                            nc.vector.tensor_mul(ptv, xjb, xib)
                        else:
                            nc.vector.tensor_mul(
                                ptv[:, 0:6], xjb[:, 0:6], xib[:, 0:6])
                            nc.gpsimd.tensor_mul(
                                ptv[:, 6:N], xjb[:, 6:N], xib[:, 6:N])
                    for gp in range(max(1, UB // 2)):
                        h2s = []
                        for half in range(2):
                            g0 = gp * 4 + half * 2
                            ps1 = eps.tile([128, 512], F32, name="ps1",
                                           tag="ps1")
                            for g in range(2):
                                gs = slice((g0 + g) * 512, (g0 + g + 1) * 512)
                                nc.tensor.matmul(
                                    ps1[64 * g:64 * g + 64, :],
                                    ew1t[:, 0:64], pt0[:, gs],
                                    start=True, stop=False,
                                    tile_position=(0, 64 * g))
                                nc.tensor.matmul(
                                    ps1[64 * g:64 * g + 64, :],
                                    ew1t[:, 64:128], pt1[:, gs],
                                    start=False, stop=True,
                                    tile_position=(0, 64 * g))
                            nc.scalar.activation(ps1[:], ps1[:], EXP,
                                                 bias=biat[:, 0:1])
                            h1 = ewk.tile([128, 512], BF16, name="h1",
                                          tag="h1")
                            nc.scalar.activation(h1[:], ps1[:], LN,
                                                 bias=biat[:, 5:6], scale=0.5)
                            h2s.append(h1)
                        ps2 = eps.tile([128, 512], F32, name="ps2", tag="ps2", bufs=2)
                        for half in range(2):
                            h1 = h2s[half]
                            for g in range(2):
                                r4 = half * 2 + g
                                nc.tensor.matmul(
                                    ps2[32 * r4:32 * r4 + 32, :],
                                    ew2t[64 * g:64 * g + 64, :],
                                    h1[64 * g:64 * g + 64, :],
                                    start=True, stop=True,
                                    tile_position=(64 * g, 32 * r4))
                        nc.scalar.activation(ps2[:], ps2[:], EXP,
                                             bias=biat[:, 1:2])
                        h2 = ewk.tile([128, 512], BF16, name="h2", tag="h2")
                        nc.scalar.activation(h2[:], ps2[:], LN,
                                             bias=biat[:, 5:6], scale=0.5)
                        ps3 = eps.tile([128, 512], F32, name="ps3", tag="ps3", bufs=2)
                        for r4 in range(4):
                            nc.tensor.matmul(
                                ps3[32 * r4:32 * r4 + 32, :],
                                ew3t[32 * r4:32 * r4 + 32, :],
                                h2[32 * r4:32 * r4 + 32, :],
                                start=True, stop=True,
                                tile_position=(32 * r4, 32 * r4))
                        # bounce + repack: row-group a = group gp*4+a holds
                        # js {2(4gp+a), +1}; cols (js:2, b:8, i:32)
                        Wsb = ewk.tile([128, 512], F32, name="Wsb",
                                       tag="Wsb")
                        nc.vector.tensor_copy(Wsb[:], ps3[:])
                        rj = 64 // UB
                        nc.sync.dma_start(
                            Wt[rj * gp:rj * gp + rj,
                               UB * N * u:UB * N * (u + 1)],
                            Wsb.rearrange("(a q) (js bi) -> a q js bi",
                                          a=4, js=16 // UB)[:, 0])
                    sps_t = sps.tile([4, UB * N], F32, name="sps_t",
                                     tag="s")
                    for lb in range(UB):
                        b = u * UB + lb
                        ls = slice(lb * N, (lb + 1) * N)
                        nc.tensor.matmul(sps_t[:, ls],
                                         G2t[:, b * 4:b * 4 + 4],
                                         Wt[:, b * N:(b + 1) * N],
                                         start=True, stop=False)
                        nc.tensor.matmul(sps_t[:, ls], eyet[:],
                                         Tn4[:, b * N:(b + 1) * N],
                                         start=False, stop=True)
                    nc.scalar.copy(TS[:, u * UB * N:(u + 1) * UB * N],
                                   sps_t[:])
                    us_ = slice(u * UB * N, (u + 1) * UB * N)
                    ub_ = slice(u * UB, (u + 1) * UB)
                    nc.sync.dma_start(tsd[:, us_], TS[:, us_])
                    TRu = ewk.tile([UB, 4 * N], F32, name="TRu", tag="TRu")
                    nc.sync.dma_start(
                        TRu.rearrange("b (i c) -> b i c", c=4),
                        tsd[:, us_].rearrange("c (b i) -> b i c", b=UB))
                    rsfu = ewk.tile([UB, N * 3], F32, name="rsfu", tag="rsfu")
                    nc.sync.dma_start(rsfu[:], rsfd[ub_, :])
                    scu = ewk.tile([UB, N], F32, name="scu", tag="scu")
                    nc.sync.dma_start(scu[:], scd[ub_, :])
                    baddu = ewk.tile([UB, 4 * N], F32, name="baddu",
                                     tag="baddu")
                    nc.sync.dma_start(baddu[:], baddd[ub_, :])
                    T2 = ewk.tile([UB, 4 * N], F32, name="T2", tag="T2")
                    nc.vector.tensor_add(T2[:], TRu[:], baddu[:])
                    T2v = T2.rearrange("b (i c) -> b i c", c=4)
                    rsv = rsfu.rearrange("b (i c) -> b i c", c=3)
                    bf = ewk.tile([UB, N * 3], F32, name="bf", tag="bf")
                    bfv = bf.rearrange("b (i c) -> b i c", c=3)
                    nc.vector.tensor_mul(
                        bfv, rsv, T2v[:, :, 3:4].broadcast_to([UB, N, 3]))
                    nc.vector.tensor_sub(bfv, bfv, T2v[:, :, 0:3])
                    scv = scu[:, :, None].broadcast_to([UB, N, 3])
                    nc.vector.tensor_mul(bfv, bfv, scv)
                    ot = ewk.tile([UB, N * 3], F32, name="ot", tag="ot")
                    otv = ot.rearrange("b (i c) -> b i c", c=3)
                    nc.vector.tensor_add(otv, rsv, bfv)
                    nc.sync.dma_start(outd[ub_, :], ot[:])

    nc.compile()
    return nc


_NC_CACHE = {}


def _get_nc(bc):
    if bc not in _NC_CACHE:
        _NC_CACHE[bc] = build_kernel(bc)
    return _NC_CACHE[bc]


def kernel(**inputs):
    in_maps = _host_prep(**inputs)
    nc = _get_nc(B // N_CORES)
    res = run_bass_kernel_spmd(nc, in_maps, core_ids=list(range(N_CORES)))
    outs = [res.results[c]["out"].reshape(B // N_CORES, N, 3)
            for c in range(N_CORES)]
    return np.concatenate(outs, axis=0).astype(np.float32)

